# revision 75
# baseline (speedup 1.0000x reference)
"""Multi-head attention (B=2, S=2048, D=1024, H=16) on 8 NeuronCores.

Sharding: 2-way batch x 4-way heads (4 heads / core). Each core computes
its 4 heads' attention output projected through its slice of Wo, giving a
partial [S, D] output (bf16); the host sums the 4 partials per batch
element and adds the bias terms (bo and the softmax-folded bv @ Wo.T).

Engine plan (v4): ACT runs the softmax exp stream (2x N=1024 activations
per iteration from PSUM, ~147us floor); PE work is packed under it.

  - Prefix is minimal: K proj (full S) then Q quarters 0/1 only, with wk
    chunked and all non-critical DMAs (wv/wo/qs/xv) sequenced behind the
    xk/xq01 streams on the sync queue.  Q quarters 2/3 are staged whole
    (xq23) and projected as 8-matmul aux-bank bursts under the loop.
  - Scores: per head-pair, two row-tiled matmuls (contraction 64) at
    tile_position (0,0)/(64,0) stream concurrently.
  - A@V packs a head pair column-wise: V_A at (0,0), V_B at (0,64) --
    the column tiles also stream concurrently.
  - Softmax row sums (PE ones-matmuls at (0, 32h)) ride a DVE add-tree:
    chunks 0-3 direct, 4-11 folded 4-into-1, 12-15 folded 2-into-1, so
    the PE stream halves and no DVE add is pending near the norm window.
  - Normalization: 1/sums as exp(-ln(sums)) on ACT (~1.4us, vs 3.4us DVE
    reciprocal whose latency head-of-line-blocked the PE FIFO at every
    quarter boundary), then rank-1 PE broadcast + DVE multiplies; norm
    runs at +18..+20 and av(qq+1) reuses the ot bank at +22.
  - Tail: quarter 3's last pairs are direct sums, tiny "warm" matmuls
    keep the HAM clock at 8/8 through the norm latencies, the final
    oproj ring alternates PSUM evacs between ACT and DVE.
"""

import ml_dtypes
import numpy as np

BF = ml_dtypes.bfloat16

import concourse.bass as bass
import concourse.mybir as mybir
import concourse.tile as tile
from concourse.bass_utils import run_bass_kernel_spmd

D_MODEL = 1024
S = 2048
B = 2
H = 4              # heads per core
DK = 64
F = H * DK         # 256 local features per core
KD = D_MODEL // 128    # 8 contraction chunks for projections
TK = S // 128          # 16 token chunks
NQ = 4                 # q quarters of 512
NITER = NQ * TK        # 64 attention iterations
LAG = 3                # A@V + sums lag behind scores (iterations)

f32 = mybir.dt.float32
bf16 = mybir.dt.bfloat16
Exp = mybir.ActivationFunctionType.Exp
Ln = mybir.ActivationFunctionType.Ln
Ident = mybir.ActivationFunctionType.Identity

PHASE_SEQ = []         # phase label per matmul, in emission order (profiling)


def _fix_matmul_waits(nc):
    """Peel extra sync waits off capped instructions onto same-engine NoOps.

    Walrus places a matmul's waits in its fused weight-load ISA struct,
    which fits only one wait; more fail codegen with "Too many sync wait
    commands". Moving all but one wait onto NoOps inserted immediately
    before the instruction on the same engine keeps identical ordering
    semantics.
    """
    import bass_rust
    n = 0
    capped = tuple(
        t for t in (getattr(mybir, name, None) for name in (
            "InstMatmult", "InstDMACopy", "InstActivation",
            "InstTensorCopy", "InstTensorTensor", "InstReciprocal",
            "InstMemset", "InstTensorScalarAffineSelect",
            "InstTensorScalarPtr", "InstTensorScalar",
            "InstTensorReduce", "InstCopy", "InstDrain",
            "InstEventSemaphore", "InstNoOp"))
        if isinstance(t, type))

    for f in nc.m.functions:
        for blk in f.blocks:
            newlist = []
            for ins in blk.instructions:
                si = ins.sync_info
                is_isa = type(ins).__name__ == "InstISA" or not isinstance(
                    ins, mybir.Instruction)
                keep = 0 if is_isa else 1
                if (si is not None and si.on_wait
                        and (isinstance(ins, capped) or is_isa)
                        and len(si.on_wait) > keep):
                    waits = list(si.on_wait)
                    keep_waits = waits[len(waits) - keep:] if keep else []
                    for w in waits[:len(waits) - keep]:
                        nop = mybir.InstNoOp(name=f"I-wfix{n}", ins=[], outs=[])
                        n += 1
                        nop.engine = ins.engine
                        nop.sync_info = bass_rust.SyncInfo(
                            on_wait=[w], on_update=[])
                        newlist.append(nop)
                    ins.sync_info = bass_rust.SyncInfo(
                        on_wait=keep_waits, on_update=list(si.on_update))
                newlist.append(ins)
            blk.instructions = newlist
    return n


def build_nc():
    nc = bass.Bass("TRN2", target_bir_lowering=False, debug=False)
    PHASE_SEQ.clear()

    def mm(phase, *args, **kwargs):
        PHASE_SEQ.append(phase)
        return nc.tensor.matmul(*args, **kwargs)

    # Host arrays pre-permuted so every DMA is per-partition contiguous.
    # xv is token-block-major so each vproj block is one contiguous DMA.
    xq = nc.dram_tensor("xq", [128, KD, S], bf16, kind="ExternalInput").ap()
    xq23 = nc.dram_tensor("xq23", [128, 2, KD, 512], bf16,
                          kind="ExternalInput").ap()
    xk = nc.dram_tensor("xk", [128, KD, S], bf16, kind="ExternalInput").ap()
    xv = nc.dram_tensor("xv", [128, TK, KD, 128], bf16,
                        kind="ExternalInput").ap()
    wq = nc.dram_tensor("wq", [128, KD, F], bf16, kind="ExternalInput").ap()
    wk = nc.dram_tensor("wk", [128, KD, F], bf16, kind="ExternalInput").ap()
    wv = nc.dram_tensor("wv", [128, KD, F], bf16, kind="ExternalInput").ap()
    wo = nc.dram_tensor("wo", [128, 2, D_MODEL], bf16, kind="ExternalInput").ap()
    bq8 = nc.dram_tensor("bq8", [128, 2], f32, kind="ExternalInput").ap()
    bk_ = nc.dram_tensor("bk_", [128, 2], f32, kind="ExternalInput").ap()
    out = nc.dram_tensor("out", [S, D_MODEL], bf16, kind="ExternalOutput").ap()

    with tile.TileContext(nc) as tc:
        with (
            tc.tile_pool(name="wpool", bufs=1) as wpool,
            tc.tile_pool(name="qkpool", bufs=1) as qkpool,
            tc.tile_pool(name="vpool", bufs=1) as vpool,
            tc.tile_pool(name="otpool", bufs=1) as otpool,
            tc.tile_pool(name="atpool", bufs=1) as atpool,
            tc.tile_pool(name="rpool", bufs=2) as rpool,
            tc.tile_pool(name="opool", bufs=2) as opool,
            tc.tile_pool(name="qspool", bufs=1) as qspool,
        ):
            # ---- persistent SBUF ----
            wq_sb = wpool.tile([128, KD, F], bf16, tag="wq")
            wk_sb = wpool.tile([128, KD, F], bf16, tag="wk")
            wv_sb = wpool.tile([128, KD, F], bf16, tag="wv")
            wo_sb = wpool.tile([128, 2, D_MODEL], bf16, tag="wo")
            bq_sb = wpool.tile([128, 2], f32, tag="bq")
            bk_sb = wpool.tile([128, 2], f32, tag="bk")
            ones_bf = wpool.tile([128, 1], bf16, tag="ones")
            ones_row = wpool.tile([128, 64], bf16, tag="ones_row")
            prime = wpool.tile([1, 1], f32, tag="prime")

            xv_sb = wpool.tile([128, TK, KD, 128], bf16, tag="xv")

            qt = qkpool.tile([128, 2, S], bf16, tag="qt")   # (Q+bq)/8
            kt = qkpool.tile([128, 2, S], bf16, tag="kt")   # K+bk
            vaug = vpool.tile([128, TK, F], bf16, tag="vaug")  # V token-major
            otn = otpool.tile([128, 2, S], bf16, tag="otn")    # normalized O.T
            at = atpool.tile([128, TK, H, 512], bf16, tag="at")  # exp(scores)

            # K is projected first, so wk leads the scalar queue -- and its
            # first half is a separate DMA so the first matmul's dependency
            # (wk[:, 0, :]) lands ~1.5us in.
            # Only wk/wq contend with the xk stream the prefix is blocked on;
            # wv/wo/qs2 ride the sync queue behind xq01 (first needed ~15
            # iterations into the loop), qs3 on the idle gpsimd queue.
            nc.scalar.dma_start(wk_sb[:, 0:1, :], wk[:, 0:1, :])
            nc.scalar.dma_start(wk_sb[:, 1:4, :], wk[:, 1:4, :])
            nc.scalar.dma_start(wk_sb[:, 4:KD, :], wk[:, 4:KD, :])
            nc.scalar.dma_start(wq_sb, wq)
            nc.gpsimd.dma_start(bq_sb, bq8)
            nc.gpsimd.dma_start(bk_sb, bk_)
            nc.gpsimd.memset(ones_bf, 1.0)
            nc.gpsimd.memset(ones_row, 1.0)
            # prime the ACT exp table set during the prefix
            nc.scalar.activation(prime, bq_sb[0:1, 0:1], Exp)

            # Q quarters 2/3 are staged so they can be projected as short
            # in-loop bursts under the exp stream (the prefix only projects
            # K and Q quarters 0/1 -- the minimum the first 16 attention
            # iterations need). One rotating buffer: q3's DMA waits on the
            # q2 burst's reads via the pool WAR, which is still ~25
            # iterations before q3 is consumed.
            qs23 = {qx: qspool.tile([128, KD, 512], bf16, tag="qs",
                                    name=f"qs{qx}")
                    for qx in (2, 3)}

            # ---- prefix: K proj (full S), then Q proj quarters 0/1 ----
            with (
                tc.tile_pool(name="xpool", bufs=6) as xpool,
                tc.tile_pool(name="xpool2", bufs=7) as xpool2,
                tc.tile_pool(name="psP", bufs=1, space="PSUM") as psP,
            ):
                ps = {(fh, half): psP.tile([128, 1024], f32,
                                           tag=f"p{fh}{half}",
                                           name=f"psK_{fh}{half}")
                      for fh in range(2) for half in range(2)}
                for kd in range(KD):
                    if kd == 0:
                        # chunk 0 lands as two halves (xca + the upper half
                        # of a regular xc tile) so the very first matmul
                        # only waits on 256 KB, not 512 KB
                        xca = qspool.tile([128, S // 2], bf16, tag="xca",
                                          name="xk0a")
                        xc = xpool.tile([128, S], bf16, tag="xc")
                        nc.sync.dma_start(xca, xk[:, 0, 0:S // 2])
                        nc.sync.dma_start(xc[:, S // 2:], xk[:, 0, S // 2:])
                    else:
                        xc = xpool.tile([128, S], bf16, tag="xc")
                        nc.sync.dma_start(xc, xk[:, kd, :])
                    for fh in range(2):
                        lhsT = wk_sb[:, kd, fh * 128:(fh + 1) * 128]
                        for qn in range(NQ):
                            if kd == 0 and qn < 2:
                                mov = xca[:, qn * 512:(qn + 1) * 512]
                            else:
                                mov = xc[:, qn * 512:(qn + 1) * 512]
                            mm("proj",
                               ps[(fh, qn // 2)][:, (qn % 2) * 512:
                                                 (qn % 2) * 512 + 512],
                               lhsT,
                               mov,
                               start=(kd == 0),
                               stop=(kd == KD - 1),
                               )
                for half in range(2):
                    cols = slice(half * 1024, half * 1024 + 1024)
                    nc.scalar.activation(
                        kt[:, 0, cols], ps[(0, half)], Ident,
                        bias=bk_sb[:, 0:1], scale=1.0)
                    nc.vector.tensor_scalar_add(
                        kt[:, 1, cols], ps[(1, half)], bk_sb[:, 1:2])

                psq = {fh: psP.tile([128, 1024], f32, tag=f"p{fh}0",
                                    name=f"psQ_{fh}")
                       for fh in range(2)}
                for kd in range(KD):
                    xcs = []
                    for qn in range(2):
                        xc = xpool2.tile([128, 512], bf16, tag="xc2",
                                         name=f"xq01_{kd}_{qn}")
                        nc.sync.dma_start(
                            xc, xq[:, kd, qn * 512:(qn + 1) * 512])
                        xcs.append(xc)
                    for fh in range(2):
                        lhsT = wq_sb[:, kd, fh * 128:(fh + 1) * 128]
                        for qn in range(2):
                            mm("proj",
                               psq[fh][:, qn * 512:qn * 512 + 512],
                               lhsT,
                               xcs[qn],
                               start=(kd == 0),
                               stop=(kd == KD - 1),
                               )
                nc.scalar.activation(
                    qt[:, 0, 0:1024], psq[0], Ident,
                    bias=bq_sb[:, 0:1], scale=0.125)
                nc.vector.tensor_scalar(
                    qt[:, 1, 0:1024], psq[1], 0.125,
                    bq_sb[:, 1:2],
                    mybir.AluOpType.mult, mybir.AluOpType.add)

            # Deferred loads on the sync queue behind the prefix's xk/xq01 --
            # sequencing them there keeps the prefix DMAs uncontended, and
            # none is needed before ~8 iterations into the loop. qs3 goes on
            # the empty gpsimd queue because its buffer-reuse wait (on the
            # q2 burst's reads) would head-of-line-block anything behind it.
            nc.sync.dma_start(wv_sb, wv)
            nc.sync.dma_start(wo_sb, wo)
            nc.sync.dma_start(qs23[2], xq23[:, 0])
            nc.gpsimd.dma_start(qs23[3], xq23[:, 1])
            for tcn in range(TK):
                nc.sync.dma_start(xv_sb[:, tcn], xv[:, tcn])

            # ---- attention: 64 iterations of (qq, kc), ACT-bound ----
            with (
                tc.tile_pool(name="psS", bufs=2, space="PSUM") as psS,
                tc.tile_pool(name="psO", bufs=1, space="PSUM") as psO,
                tc.tile_pool(name="psR", bufs=1, space="PSUM") as psR,
                tc.tile_pool(name="psX", bufs=1, space="PSUM") as psX,
            ):
                ot_tiles = {}
                sums_tiles = {}
                rst_tiles = {}
                rb_ps_tiles = {}
                stmp_tiles = {}

                def emit_vproj(tcn):
                    pv = psX.tile([128, 512], f32, tag="aux", name=f"vp{tcn}")
                    for kd in range(KD):
                        mm("vproj",
                           pv[:, 0:F], xv_sb[:, tcn, kd, :],
                           wv_sb[:, kd, :],
                           start=(kd == 0), stop=(kd == KD - 1))
                    nc.vector.tensor_copy(vaug[:, tcn, :], pv[:, 0:F])

                def emit_qburst(qx, fh):
                    # Q quarter 2/3 projection burst from the staged slice:
                    # 8 chained matmuls into the aux bank, bias on DVE.
                    pq = psX.tile([128, 512], f32, tag="aux",
                                  name=f"qb{qx}_{fh}")
                    for kd in range(KD):
                        mm("proj",
                           pq, wq_sb[:, kd, fh * 128:(fh + 1) * 128],
                           qs23[qx][:, kd, :],
                           start=(kd == 0), stop=(kd == KD - 1))
                    nc.vector.tensor_scalar(
                        qt[:, fh, qx * 512:(qx + 1) * 512], pq, 0.125,
                        bq_sb[:, fh:fh + 1],
                        mybir.AluOpType.mult, mybir.AluOpType.add)

                def emit_scores(qq, kc):
                    q0 = qq * 512
                    k0 = kc * 128
                    sts = []
                    for pair in range(2):
                        st = psS.tile([128, 2, 512], f32, tag="st",
                                      name=f"st{qq}_{kc}_{pair}")
                        sts.append(st)
                        for r in (0, 64):
                            mm("scores",
                               st[:, r // 64, :],
                               kt[r:r + 64, pair, k0:k0 + 128],
                               qt[r:r + 64, pair, q0:q0 + 512],
                               start=True, stop=True,
                               tile_position=(r, 0))
                    for pair in range(2):
                        nc.scalar.activation(
                            at[:, kc, 2 * pair:2 * pair + 2, :], sts[pair], Exp)

                def emit_av(qq, kc):
                    if kc == 0:
                        ot_tiles[qq] = psO.tile([128, 2, 512], f32, tag="ot",
                                                name=f"ot{qq}")
                    ot = ot_tiles[qq]
                    for pair in range(2):
                        for ab in range(2):
                            h = 2 * pair + ab
                            mm("av",
                               ot[64 * ab:64 * ab + 64, pair, :],
                               vaug[:, kc, 64 * h:64 * h + 64],
                               at[:, kc, h, :],
                               start=(kc == 0), stop=(kc == TK - 1),
                               tile_position=(0, 64 * ab))

                def emit_sums_direct(qq, kc, stop=False):
                    # chunks 0-3 of each quarter keep the direct per-chunk
                    # row-sum stream: no DVE dependency, so the quarter-
                    # boundary reciprocal never head-of-line-blocks the PE
                    # FIFO through a pending pair-add.
                    if kc == 0:
                        sums_tiles[qq] = psR.tile([128, 512], f32, tag="sums",
                                                  name=f"sums{qq}")
                    sums = sums_tiles[qq]
                    for h in range(H):
                        mm("sums",
                           sums[32 * h:32 * h + 1, :],
                           ones_bf,
                           at[:, kc, h, :],
                           start=(kc == 0), stop=stop,
                           tile_position=(0, 32 * h),
                           skip_group_check=True)

                def emit_sums_add(qq, p):
                    # level-2 tree: DVE-add the kc pair (2p, 2p+1) so each
                    # PE row-sum stream covers two chunks (halves sums mms)
                    tmp = rpool.tile([128, H, 512], bf16, tag="stmp",
                                     name=f"stmp{qq}_{p}")
                    for h in range(H):
                        nc.vector.tensor_add(tmp[:, h, :],
                                             at[:, 2 * p, h, :],
                                             at[:, 2 * p + 1, h, :])
                    stmp_tiles[(qq, p)] = tmp

                def emit_sums_add2(qq, p):
                    # level-3: fold the two pair-sums (p, p+1) into one tile
                    # so the PE stream covers four chunks
                    a = stmp_tiles.pop((qq, p))
                    b = stmp_tiles[(qq, p + 1)]
                    for h in range(H):
                        nc.vector.tensor_add(b[:, h, :], a[:, h, :],
                                             b[:, h, :])

                def emit_sums_mm(qq, p):
                    sums = sums_tiles[qq]
                    tmp = stmp_tiles.pop((qq, p))
                    for h in range(H):
                        mm("sums",
                           sums[32 * h:32 * h + 1, :],
                           ones_bf,
                           tmp[:, h, :],
                           start=False, stop=(p == TK // 2 - 1),
                           tile_position=(0, 32 * h),
                           skip_group_check=True)

                def emit_norm_a(qq):
                    # 1/sums as exp(-ln(sums)) on ACT: ~1.4us vs the 3.4us
                    # DVE iterative divide, whose latency head-of-line-blocked
                    # the bcast matmuls (and so the whole PE FIFO) at every
                    # quarter boundary. Ln and Exp share one table set.
                    sums = sums_tiles.pop(qq)
                    lns = rpool.tile([128, 512], f32, tag="rst",
                                     name=f"lns{qq}")
                    nc.scalar.activation(lns, sums, Ln)
                    rstb = rpool.tile([128, 512], bf16, tag="rstb",
                                      name=f"rstb{qq}")
                    nc.scalar.activation(rstb, lns, Exp, scale=-1.0)
                    rst_tiles[qq] = rstb

                def emit_norm_b(qq):
                    # rank-1 PE broadcast: rb[64ab+j, q] = rst[32h, q]
                    rst = rst_tiles[qq]
                    for pair in range(2):
                        # allocate from the aux bank (not psR): otherwise the
                        # next quarter's sums matmuls chain behind the whole
                        # recip->cast->bcast->copy sequence and stall the PE
                        # FIFO at every quarter boundary (HAM re-throttle).
                        rb_ps = psX.tile([128, 512], f32, tag="aux",
                                         name=f"rbp{qq}_{pair}")
                        for ab in range(2):
                            h = 2 * pair + ab
                            mm("bcast",
                               rb_ps[64 * ab:64 * ab + 64, :],
                               ones_row[32 * h:32 * h + 1, :],
                               rst[32 * h:32 * h + 1, :],
                               start=True, stop=True,
                               tile_position=(32 * h, 64 * ab))
                        rb_ps_tiles[(qq, pair)] = rb_ps

                def emit_norm_c(qq, tail=False):
                    # tail=True: the pair-0 rb copy moves to ACT (idle after
                    # the last exp) so both copies run concurrently and the
                    # DVE queue only carries the otn muls.
                    q0 = qq * 512
                    ot = ot_tiles.pop(qq)
                    rst_tiles.pop(qq)
                    for pair in range(2):
                        rb_ps = rb_ps_tiles.pop((qq, pair))
                        rb = rpool.tile([128, 512], f32, tag="rb",
                                        name=f"rb{qq}_{pair}")
                        if tail and pair == 0:
                            nc.scalar.activation(rb, rb_ps, Ident)
                        else:
                            nc.vector.tensor_copy(rb, rb_ps)
                        for ab in range(2):
                            nc.vector.tensor_mul(
                                otn[64 * ab:64 * ab + 64, pair, q0:q0 + 512],
                                ot[64 * ab:64 * ab + 64, pair, :],
                                rb[64 * ab:64 * ab + 64, :])

                ostage_tiles = {}

                def emit_oproj_half(tcn, nh, ring=False):
                    # ring=True (tail only): the scores ring is free after
                    # the last exp, so use its 2-deep rotation instead of
                    # ping-ponging the single aux bank; the PSUM->SBUF casts
                    # alternate between ACT (idle after the last exp) and DVE
                    # so the PE never waits on a single engine's drain.
                    if nh == 0:
                        ostage_tiles[tcn] = opool.tile(
                            [128, D_MODEL], bf16, tag="ostage",
                            name=f"os{tcn}")
                    ostage = ostage_tiles[tcn]
                    if ring:
                        pdt = psS.tile([128, 2, 512], f32, tag="st",
                                       name=f"opr{tcn}_{nh}")
                        pd = pdt[:, 0, :]
                    else:
                        pd = psX.tile([128, 512], f32, tag="aux",
                                      name=f"op{tcn}_{nh}")
                    for fc in range(2):
                        mm("oproj",
                           pd,
                           otn[:, fc, tcn * 128:(tcn + 1) * 128],
                           wo_sb[:, fc, nh * 512:(nh + 1) * 512],
                           start=(fc == 0), stop=(fc == 1))
                    dst = ostage[:, nh * 512:(nh + 1) * 512]
                    if ring and nh == 1:
                        nc.scalar.activation(dst, pd, Ident)
                    else:
                        nc.vector.tensor_copy(dst, pd)
                    if ring and tcn == TK - 1:
                        # last chunk: per-half DMA so the final HBM write's
                        # ~2us completion receipt starts one cast earlier
                        nc.sync.dma_start(
                            out[tcn * 128:(tcn + 1) * 128,
                                nh * 512:(nh + 1) * 512], dst)
                        if nh == 1:
                            ostage_tiles.pop(tcn)
                    elif nh == 1:
                        ostage_tiles.pop(tcn)
                        nc.sync.dma_start(
                            out[tcn * 128:(tcn + 1) * 128, :], ostage)

                # schedules: norm stages and oproj chunks per iteration.
                # The reciprocal (norm_a) runs 3 iterations before the
                # broadcast matmuls (norm_b) so its ~3.4us DVE latency never
                # blocks the PE FIFO (which previously re-throttled the HAM
                # clock at every quarter boundary).
                # norm_a on ACT finishes ~+19.3, so norm_b/c can run at +20:
                # the DVE otn muls then clear the queue two iterations before
                # av(qq+1, 0) allocates the ot bank at +22.
                norm_a = {16 * qq + 18: qq for qq in range(3)}
                norm_bc = {16 * qq + 20: qq for qq in range(3)}
                # oproj halves start 2 iterations after the quarter's norm_c
                # so the first chunk's matmuls never wait on the otn muls.
                sched_oproj = {}
                for i in range(12):
                    base = 16 * (i // 4)
                    off = (26 + 3 * (i % 4)) if i < 8 else (24 + 2 * (i % 4))
                    sched_oproj[base + off] = (i, 0)
                    sched_oproj[base + off + 1] = (i, 1)
                # Q quarter 2/3 bursts in aux-bank gaps of the vproj/norm/
                # oproj calendar, well before scores needs them (t=32/48).
                sched_qburst = {18: (2, 0), 19: (2, 1), 40: (3, 0),
                                44: (3, 1)}
                # sums pairs (qq, p>=2): DVE add one iteration after exp of
                # the odd chunk; PE stream one more iteration later so the
                # row-sum matmuls never head-of-line-block the PE FIFO on the
                # add. p=0,1 (chunks 0-3) go direct with the av spread, which
                # keeps the DVE queue clear of adds in the [+17,+21] norm
                # window around the reciprocal. Quarter 3's last pair is also
                # direct so the post-loop norm chain never waits on a DVE add.
                # chunks 4-11 get a third tree level (4 chunks per PE
                # stream); chunks 12-15 stay at level 2 so no DVE add is in
                # flight near the quarter-boundary norm window.
                sched_sadd = {}
                sched_sadd2 = {}
                sched_smm = {}
                for qq in range(4):
                    last_p = 6 if qq == 3 else 7
                    for p in range(2, last_p + 1):
                        sched_sadd[16 * qq + 2 * p + 2] = (qq, p)
                    for p in (2, 4):
                        sched_sadd2[16 * qq + 2 * p + 5] = (qq, p)
                    sched_smm[16 * qq + 10] = (qq, 3)
                    sched_smm[16 * qq + 14] = (qq, 5)
                    for p in ((6, 7) if qq < 3 else (6,)):
                        sched_smm[16 * qq + 2 * p + 3] = (qq, p)
                # A@V spread: chunks 0-5 arrive two per iteration so the
                # previous quarter's norm chain keeps its 3-iteration window
                # without a 4-chunk burst stalling the exp stream.
                av_spread = {3: (0, 1), 4: (2, 3), 5: (4, 5)}

                # vproj(0) runs in the prefix->loop transition gap, where
                # the PE otherwise idles ~1.5us waiting on the Q01 epilogue
                # (ACT/DVE bias adds) that gates the first scores matmul.
                emit_vproj(0)

                for t in range(NITER):
                    qq, kc = divmod(t, TK)
                    emit_scores(qq, kc)
                    if t in sched_sadd:
                        emit_sums_add(*sched_sadd[t])
                    if t in norm_bc:
                        emit_norm_b(norm_bc[t])
                        emit_norm_c(norm_bc[t])
                    if 1 <= t < TK:
                        emit_vproj(t)
                    if t in sched_qburst:
                        emit_qburst(*sched_qburst[t])
                    if t in sched_oproj:
                        emit_oproj_half(*sched_oproj[t])
                    s = t - LAG
                    if s >= 0:
                        r = s % TK
                        if r in av_spread:
                            for rr in av_spread[r]:
                                emit_av(s // TK, rr)
                                if rr < 4:
                                    emit_sums_direct(s // TK, rr)
                        elif r > 5:
                            emit_av(s // TK, r)
                    if t in sched_sadd2:
                        emit_sums_add2(*sched_sadd2[t])
                    if t in sched_smm:
                        emit_sums_mm(*sched_smm[t])
                    if t in norm_a:
                        emit_norm_a(norm_a[t])

                emit_av(3, 13)
                emit_av(3, 14)
                emit_sums_direct(3, 14)
                emit_av(3, 15)
                emit_sums_direct(3, 15, stop=True)
                sums3 = sums_tiles[3]
                emit_norm_a(3)

                def emit_warm(i, moving):
                    # tiny matmul whose only job is PE-activity during the
                    # tail's norm latencies, so the HAM clock never drops
                    # before the final oproj burst (cold costs it ~1.6x).
                    mm("warm", sums3[0:1, 64 * i:64 * i + 64],
                       ones_bf[0:1, :], moving,
                       start=True, stop=True, skip_group_check=True)

                emit_warm(0, at[0:1, 15, 0, 0:64])
                emit_warm(1, rst_tiles[3][0:1, 0:64])
                emit_norm_b(3)
                emit_warm(2, rst_tiles[3][0:1, 64:128])
                emit_norm_c(3, tail=True)
                emit_warm(3, otn[0:1, 0, 1536:1600])
                for i in range(12, TK):
                    emit_oproj_half(i, 0, ring=True)
                    emit_oproj_half(i, 1, ring=True)

    _fix_matmul_waits(nc)
    return nc


_NC = None


def _get_nc():
    global _NC
    if _NC is None:
        _NC = build_nc()
    return _NC


def _chunked(xT):
    # [1024, S] -> [128, 8, S] with [p, c, t] = xT[c*128+p, t]
    return np.ascontiguousarray(
        xT.reshape(KD, 128, -1).transpose(1, 0, 2)).astype(BF)


def _chunked_tb(xT):
    # [1024, S] -> [128, TK, KD, 128]: token-block-major so each vproj
    # block is one contiguous per-partition DMA
    c = xT.reshape(KD, 128, TK, 128)           # [kd, p, tb, tok]
    return np.ascontiguousarray(c.transpose(1, 2, 0, 3)).astype(BF)


def make_in_maps(q, k, v, Wq, bq, Wk, bk, Wv, bv, Wo, bo):
    q = np.asarray(q, np.float32)
    k = np.asarray(k, np.float32)
    v = np.asarray(v, np.float32)
    xh = {}
    for b in range(B):
        xqc = _chunked(q[b].T)
        xh[("q", b)] = xqc
        xh[("q23", b)] = np.ascontiguousarray(
            xqc[:, :, 1024:].reshape(128, KD, 2, 512).transpose(0, 2, 1, 3))
        xh[("k", b)] = _chunked(k[b].T)
        xh[("v", b)] = _chunked_tb(v[b].T)
    in_maps = []
    for c in range(8):
        b, g = divmod(c, 4)
        sl = slice(F * g, F * (g + 1))
        woT = np.asarray(Wo, np.float32)[:, sl].T  # [256, 1024]
        in_maps.append({
            "xq": xh[("q", b)],
            "xq23": xh[("q23", b)],
            "xk": xh[("k", b)],
            "xv": xh[("v", b)],
            "wq": _chunked(np.asarray(Wq, np.float32)[sl, :].T),
            "wk": _chunked(np.asarray(Wk, np.float32)[sl, :].T),
            "wv": _chunked(np.asarray(Wv, np.float32)[sl, :].T),
            "wo": np.ascontiguousarray(
                woT.reshape(2, 128, D_MODEL).transpose(1, 0, 2)).astype(BF),
            "bq8": np.ascontiguousarray(
                (np.asarray(bq, np.float32)[sl] / 8.0).reshape(2, 128).T),
            "bk_": np.ascontiguousarray(
                np.asarray(bk, np.float32)[sl].reshape(2, 128).T),
        })
    return in_maps


def gather(results, bv, bo, Wo):
    const = (np.asarray(bo, np.float64)
             + np.asarray(bv, np.float64) @ np.asarray(Wo, np.float64).T)
    out = np.zeros((B, S, D_MODEL), np.float32)
    for c in range(8):
        out[c // 4] += np.asarray(results[c]["out"], dtype=np.float32)
    out += const.astype(np.float32)
    return out


def kernel(q, k, v, Wq, bq, Wk, bk, Wv, bv, Wo, bo):
    nc = _get_nc()
    in_maps = make_in_maps(q, k, v, Wq, bq, Wk, bk, Wv, bv, Wo, bo)
    res = run_bass_kernel_spmd(nc, in_maps, list(range(8))).results
    return gather(res, bv, bo, Wo)

